# revision 41
# baseline (speedup 1.0000x reference)
"""Trainium2 Bass kernel v4 for nn_CHGANSimplified (sparse graph attention).

Math (per batch b, time t):
  enh = x + type_embed[parity(n)]
  Q/K/V = enh @ W*.T + b*          (4 heads, head dim 32)
  S_h = (Q_h K_h^T)/sqrt(32) + edge_bias ; masked where adj==0 & ~eye
  out = LN(concat_h(softmax(S_h) V_h) @ Wo.T + bo + x)

v4 vs v3 (engine rebalance around the exp bottleneck; 120486 -> 91866ns
under the CoreSim cost model, rel err 2.5e-3 on HW):
  - type-embed folded host-side into enh: no qta/kta/vtab device adds;
    qt/kt are single [128,N] tiles (score matmuls pass explicit
    tile_position=(32h,0) so head 3 can sit at partition 96)
  - scores pre-scaled by C1=2^7/ln2 (folded into Wq with 1/sqrt(32)), so
    exp splits per m-tile route:
      'S': one DVE tensor_tensor int16(st + maskc2) -> Schraudolph exp
           bits reinterpreted as bf16 (mask folded into the add; fp32->
           int16 saturation maps masked entries to -0.0)
      'P': ACT exp(scale=1/C1) + Pool multiplicative mask (no PE inject:
           keeping inject matmuls off the score-PSUM ring shortens its
           critical chain, and Pool had the slack)
    The exact-exp path carries a 1.0402 gain matching Schraudolph's mean
    bias so mixed tiles don't tilt the softmax.  Routes are interleaved
    per slot (S/P alternating) so the DVE/ACT readers of the score ring
    run concurrently on its even/odd lanes.
  - PSUM: 3 x 2-bank score ring (the critical resource: slot period ~
    sum of per-tile reader costs over the lane count) + a SINGLE 1-bank
    AV accumulator -- av(s) accumulates during slot s+1, is normed at the
    s+1/s+2 boundary, and the bank is reused by av(s+2); AV chunks are
    emitted one m-iteration late so PE never blocks on the boundary norm
    + 1 bank for proj/vps/tp/op transients (pair-0 proj and the last
    pair's epilogue borrow the then-idle score ring instead)
  - the LN apply runs on Pool; q-proj PSUM copies on ACT (Copy shares the
    Exp table, so no table swaps); k-proj/v/y/tp readouts on DVE
  - boundary work (epilogue, next-pair qk/v) is split into small pieces
    pumped one-per-m-iteration into the slot loop (next-pair qk/v jump
    the queue) so in-order engine queues never starve
"""

import os
import sys

sys.path.insert(0, "/opt/trn_rl_repo")

from contextlib import ExitStack

import ml_dtypes
import numpy as np

import concourse.bass as bass
import concourse.tile as tile
from concourse import bacc, mybir
from concourse.bass_utils import run_bass_kernel_spmd

B, N, T, D, H, DH = 2, 1024, 12, 128, 4, 32
NCORES = 8
PAIRS = [(b, t) for b in range(B) for t in range(T)]
PER_CORE = len(PAIRS) // NCORES  # 3
EPS = 1e-5
NTILE = N // 128  # 8

MM_DT, MM_NP = mybir.dt.bfloat16, ml_dtypes.bfloat16
F32 = mybir.dt.float32
I32 = mybir.dt.int32
I16 = mybir.dt.int16
AF = mybir.ActivationFunctionType
ALU = mybir.AluOpType

C1 = 184.66496523378732  # 2^7 / ln 2
INV_C1 = 1.0 / C1
C2_KEEP = 16256.0  # 127 << 7 (bf16 exponent bias bits)
C2_MASK = -49152.0  # saturates int16 -> 0x8000 -> bf16 -0.0
CORR = 1.0402265  # mean schraudolph/exact ratio; applied to exact paths
MASKA_MASK = -70656.0  # C1 * -382.6 ; exp -> 0
QMAGIC = 0x5F3759DF

# per-slot m-tile routes: S=DVE schraudolph, I=PE inject + ACT exp,
# P=ACT exp + Pool mult, V=ACT exp + DVE mult
# interleaved so the three reader engines (DVE for S, ACT for I/P, Pool
# after ACT for P) overlap under the 2-buffer score-PSUM pacing; per-head
# patterns because slots (it,3) are DVE-lean (epilogue(it) not ready yet)
# and (it,1)/(it,2) are DVE-rich (epilogue + next-pair qk/v copies)
PATH0 = os.environ.get("BASSK_PATH0", "SPSPPSPP")  # 3S
PATH1 = os.environ.get("BASSK_PATH1", "SPPPSPPP")  # 2S
PATH2 = os.environ.get("BASSK_PATH2", "SPPPSPPP")  # 2S
PATH3 = os.environ.get("BASSK_PATH3", "SPSPPSPP")  # 3S
PATL = os.environ.get("BASSK_PATL", "SPSPSPPS")
BN_POOL = int(os.environ.get("BASSK_BN_POOL", "0"))  # gpsimd has no bn_stats
LN_POOL = int(os.environ.get("BASSK_LN_POOL", "1"))
OT_ACT = int(os.environ.get("BASSK_OT_ACT", "0"))  # ot copies on ACT

NSLOTS = PER_CORE * H


def _slot_pattern(s):
    if s == NSLOTS - 1:
        return PATL
    return (PATH0, PATH1, PATH2, PATH3)[s % H]


def _needed_masks():
    c2, ka, km = set(), set(), set()
    for s in range(NSLOTS):
        p = _slot_pattern(s)
        for m, r in enumerate(p):
            if r == "S":
                c2.add(m)
            elif r == "I":
                ka.add(m)
            elif r == "H":
                c2.add(m)
                ka.add(m)
            else:
                km.add(m)
    return sorted(c2), sorted(ka), sorted(km)


LAST_RESULTS = None  # BassKernelResults of the most recent run (for test.py)


def _build_nc(ln_trivial: bool, bias_trivial: bool = True):
    nc = bacc.Bacc()
    need_c2, need_a, need_m = _needed_masks()

    enh_d = nc.dram_tensor("enh", [PER_CORE, 128, N], MM_DT, kind="ExternalInput")
    xpb_d = nc.dram_tensor("xpb", [PER_CORE, N, D], F32, kind="ExternalInput")
    wq_d = nc.dram_tensor("wq", [D, D], MM_DT, kind="ExternalInput")
    wk_d = nc.dram_tensor("wk", [D, D], MM_DT, kind="ExternalInput")
    wv_d = nc.dram_tensor("wv", [D, D], MM_DT, kind="ExternalInput")
    wo_d = nc.dram_tensor("wo", [D, D], MM_DT, kind="ExternalInput")
    bqk_d = nc.dram_tensor("bqk", [128, 2], F32, kind="ExternalInput")
    bvb_d = nc.dram_tensor("bvb", [128, D], F32, kind="ExternalInput")
    lng_d = nc.dram_tensor("lng", [128, D], F32, kind="ExternalInput")
    lnb_d = nc.dram_tensor("lnb", [128, D], F32, kind="ExternalInput")
    id_d = nc.dram_tensor("ident", [128, 128], MM_DT, kind="ExternalInput")
    # masks, transposed (m, nq); only needed m-tiles are loaded
    maskc2_d = nc.dram_tensor("maskc2", [N, N], MM_DT, kind="ExternalInput")
    maska_d = nc.dram_tensor("maska", [N, N], MM_DT, kind="ExternalInput")
    maskm_d = nc.dram_tensor("maskm", [N, N], MM_DT, kind="ExternalInput")
    out_d = nc.dram_tensor("out", [PER_CORE, N, D], F32, kind="ExternalOutput")

    with tile.TileContext(nc) as tc, ExitStack() as ctx:
        const = ctx.enter_context(tc.tile_pool(name="const", bufs=1))
        work = ctx.enter_context(tc.tile_pool(name="work", bufs=2))
        expp = ctx.enter_context(tc.tile_pool(name="expp", bufs=16))
        # 8 PSUM banks: scores 3 x 2 banks (3-lane readout ring) + 1 AV
        # accumulator bank (av(s) is normed at the slot boundary before
        # av(s+1) starts) + 1 bank for proj/vps/tp/op transients
        pst = ctx.enter_context(tc.tile_pool(name="pst", bufs=3, space="PSUM"))
        pav = ctx.enter_context(tc.tile_pool(name="pav", bufs=1, space="PSUM"))
        ppv = ctx.enter_context(tc.tile_pool(name="ppv", bufs=1, space="PSUM"))

        # ---- constants ----
        # SP (sync) queue: weights + S/I masks gating the first scores;
        # Pool queue: enh/xpb inputs + maskm.
        wq_sb = const.tile([D, D], MM_DT)
        nc.sync.dma_start(wq_sb, wq_d[:, :])
        wk_sb = const.tile([D, D], MM_DT)
        nc.sync.dma_start(wk_sb, wk_d[:, :])
        id_sb = const.tile([128, 128], MM_DT)
        nc.sync.dma_start(id_sb, id_d[:, :])
        bqk_sb = const.tile([128, 2], F32)
        nc.sync.dma_start(bqk_sb, bqk_d[:, :])

        # pre-load the Exp table while DMAs are in flight so the first real
        # exp doesn't pay the table switch
        tiny = const.tile([128, 1], F32)
        nc.vector.memset(tiny, 0.0)
        tiny2 = const.tile([128, 1], MM_DT)
        nc.scalar.activation(tiny2, tiny, AF.Exp)

        # masks in first-use (m) order so slot 0 is never DMA-gated
        maskc2_sb = {}
        maska_sb = {}
        maskm_sb = {}
        for m in range(NTILE):
            if m in need_c2:
                t = const.tile([128, N], MM_DT, name=f"mc2_{m}", tag=f"mc2_{m}")
                nc.sync.dma_start(t, maskc2_d[m * 128 : (m + 1) * 128, :])
                maskc2_sb[m] = t
            if m in need_a:
                t = const.tile([128, N], MM_DT, name=f"ma_{m}", tag=f"ma_{m}")
                nc.sync.dma_start(t, maska_d[m * 128 : (m + 1) * 128, :])
                maska_sb[m] = t

        def load_x(it):
            enh_sb = work.tile([128, N], MM_DT, name=f"enh{it}", tag="enh")
            nc.gpsimd.dma_start(enh_sb, enh_d[it])
            xpb_sb = work.tile([128, NTILE, D], F32, name=f"xpb{it}", tag="xpb")
            nc.gpsimd.dma_start(xpb_sb, xpb_d[it].rearrange("(q p) d -> p q d", p=128))
            return enh_sb, xpb_sb

        enh0_sb = work.tile([128, N], MM_DT, name="enh0", tag="enh")
        nc.gpsimd.dma_start(enh0_sb, enh_d[0])
        for m in need_m:
            t = const.tile([128, N], MM_DT, name=f"mm_{m}", tag=f"mm_{m}")
            nc.gpsimd.dma_start(t, maskm_d[m * 128 : (m + 1) * 128, :])
            maskm_sb[m] = t
        xpb0_sb = work.tile([128, NTILE, D], F32, name="xpb0", tag="xpb")
        nc.gpsimd.dma_start(xpb0_sb, xpb_d[0].rearrange("(q p) d -> p q d", p=128))
        x_sb = {0: (enh0_sb, xpb0_sb)}

        wv_sb = const.tile([D, D], MM_DT)
        nc.sync.dma_start(wv_sb, wv_d[:, :])
        bvb_sb = const.tile([128, D], F32)
        nc.sync.dma_start(bvb_sb, bvb_d[:, :])
        wo_sb = const.tile([D, D], MM_DT)
        nc.sync.dma_start(wo_sb, wo_d[:, :])
        lng_sb = const.tile([128, D], F32)
        nc.sync.dma_start(lng_sb, lng_d[:, :])
        lnb_sb = const.tile([128, D], F32)
        nc.sync.dma_start(lnb_sb, lnb_d[:, :])
        half_sb = const.tile([128, 1], F32)
        nc.vector.memset(half_sb, 0.5)
        c32_sb = const.tile([128, 1], F32)
        nc.vector.memset(c32_sb, 1.5)
        magic_sb = const.tile([128, 1], I32)
        nc.vector.memset(magic_sb, QMAGIC)

        # PE p-state warmup: harmless matmuls on a zeroed tile so the clock
        # ramp (full speed after 3us busy) is done before the real QK work.
        warm_sb = const.tile([128, 512], MM_DT)
        nc.vector.memset(warm_sb, 0.0)
        wps = pav.tile([16, 512], F32, name="warm", tag="av")
        for _ in range(6):
            nc.tensor.matmul(wps, warm_sb[:, 0:16], warm_sb, start=True, stop=True)

        # ---- per-pair state ----
        qk = {}  # it -> (qt, kt) [128, N] bf16
        vaugs = {}  # it -> [8 x (128, H, DH+1) tiles]
        es_all = {}  # (it, h) -> [8 x es tiles]
        av_all = {}  # (it, h) -> psum tile (128, NTILE, DH+1)
        onat = {}  # it -> (128, NTILE, D) bf16

        def qk_pieces(it):
            enh_sb, _ = x_sb[it]
            tiles = {}
            # pair 0's projections run before any scores: use the empty
            # score ring so the 1-buf ppv ring doesn't serialize startup
            qpool, qtag = (pst, "st") if it == 0 else (ppv, "pv")

            def piece(j):
                js = slice(j * 512, (j + 1) * 512)
                for nm, w_sb, c in (("q", wq_sb, 0), ("k", wk_sb, 1)):
                    if j == 0:
                        tiles[nm] = work.tile(
                            [128, N], MM_DT, name=f"{nm}t{it}", tag=f"{nm}t"
                        )
                    t = tiles[nm]
                    ps = qpool.tile([128, 512], F32, name=f"ps{nm}{it}_{j}", tag=qtag)
                    nc.tensor.matmul(ps, w_sb, enh_sb[:, js], start=True, stop=True)
                    if bias_trivial and nm == "q":
                        nc.scalar.activation(t[:, js], ps, AF.Copy)
                    else:
                        nc.vector.tensor_scalar(
                            t[:, js], ps, bqk_sb[:, c : c + 1], None, op0=ALU.add
                        )

            def fin():
                qk[it] = (tiles["q"], tiles["k"])

            return [
                lambda: piece(0),
                lambda: piece(1),
                fin,
            ]

        def v_pieces(it):
            enh_sb, _ = x_sb[it]
            va = []
            vaugs[it] = va

            def piece(m):
                vps = ppv.tile([128, D], F32, name=f"vps{it}_{m}", tag="pv")
                nc.tensor.matmul(
                    vps, enh_sb[:, m * 128 : (m + 1) * 128], wv_sb, start=True, stop=True
                )
                vt = work.tile(
                    [128, H, DH + 1], MM_DT, name=f"vaug{it}_{m}", tag=f"vaug{m}"
                )
                nc.gpsimd.memset(vt[:, :, DH : DH + 1], 1.0)
                nc.vector.tensor_add(
                    vt[:, :, 0:DH],
                    vps.rearrange("p (h d) -> p h d", h=H),
                    bvb_sb.rearrange("p (h d) -> p h d", h=H),
                )
                va.append(vt)

            return [(lambda m=m: piece(m)) for m in range(NTILE)]

        def emit_qk(it):
            for p in qk_pieces(it):
                p()

        def emit_v(it):
            for p in v_pieces(it):
                p()

        def emit_score_m(it, h, m, rt):
            """scores + exp + mask for one (pair, head, m-tile); rt routes.

            'H' splits the tile: js0 half read by a DVE schraudolph TT while
            js1 (additive mask PE-injected) is read by ACT exp — the two
            half-readers run in parallel, shortening the score-PSUM ring's
            critical chain."""
            qt, kt = qk[it]
            po = 32 * h
            st = pst.tile([128, N], F32, name=f"st{it}_{h}_{m}", tag="st")
            inj = (1,) if rt == "H" else (0, 1) if rt == "I" else ()
            for j in inj:
                nc.tensor.matmul(
                    st[:, j * 512 : (j + 1) * 512],
                    id_sb,
                    maska_sb[m][:, j * 512 : (j + 1) * 512],
                    start=True,
                    stop=False,
                )
            for j in range(2):
                nc.tensor.matmul(
                    st[:, j * 512 : (j + 1) * 512],
                    kt[po : po + 32, m * 128 : (m + 1) * 128],
                    qt[po : po + 32, j * 512 : (j + 1) * 512],
                    start=(j not in inj),
                    stop=True,
                    tile_position=(po, 0),
                )
            e = expp.tile([128, N], MM_DT, name=f"e{it}_{h}_{m}", tag="expst")
            if rt == "S":
                nc.vector.tensor_tensor(e.bitcast(I16), st, maskc2_sb[m], ALU.add)
            elif rt == "H":
                nc.vector.tensor_tensor(
                    e.bitcast(I16)[:, 0:512], st[:, 0:512],
                    maskc2_sb[m][:, 0:512], ALU.add,
                )
                nc.scalar.activation(
                    e[:, 512:N], st[:, 512:N], AF.Exp, scale=INV_C1
                )
            else:
                nc.scalar.activation(e, st, AF.Exp, scale=INV_C1)
                if rt == "P":
                    nc.gpsimd.tensor_mul(e, e, maskm_sb[m])
                elif rt == "V":
                    nc.vector.tensor_mul(e, e, maskm_sb[m])
            return e

        def emit_av_chunk(it, h, q):
            """8 accumulating AV matmuls for q-block q of (pair, head)."""
            if q == 0:
                av_all[(it, h)] = pav.tile(
                    [128, NTILE, DH + 1], F32, name=f"av{it}_{h}", tag="av"
                )
            av = av_all[(it, h)]
            es = es_all[(it, h)]
            va = vaugs[it]
            for m in range(NTILE):
                nc.tensor.matmul(
                    av[:, q, :],
                    es[m][:, q * 128 : (q + 1) * 128],
                    va[m][:, h, :],
                    start=(m == 0),
                    stop=(m == NTILE - 1),
                )

        def emit_norm(it, h):
            """batched reciprocal + normalize for slot (it, h)."""
            if h == 0:
                onat[it] = work.tile([128, NTILE, D], MM_DT, name=f"on{it}", tag="onat")
            av = av_all[(it, h)]
            rec = work.tile([128, NTILE], F32, name=f"rec{it}_{h}", tag="rec", bufs=4)
            nc.vector.reciprocal(rec, av[:, :, DH])
            nc.vector.tensor_mul(
                onat[it][:, :, h * DH : (h + 1) * DH],
                av[:, :, 0:DH],
                rec[:, :, None].to_broadcast((128, NTILE, DH)),
            )

        def epi_pieces(it):
            _, xpb_sb = x_sb[it]
            bn = nc.gpsimd if BN_POOL else nc.vector
            lne = nc.gpsimd if LN_POOL else nc.vector
            act_cp = OT_ACT or it == PER_CORE - 1
            ot = work.tile([128, N], MM_DT, name=f"ot{it}", tag="ot")
            y = work.tile([128, NTILE, D], F32, name=f"y{it}", tag="y")
            mv = work.tile([128, NTILE, 2], F32, name=f"mv{it}", tag="mv")
            oall = work.tile([128, NTILE, D], F32, name=f"oall{it}", tag="oall")
            yq = work.tile([128, NTILE], F32, name=f"yq{it}", tag="yq")

            epool = pst if it == PER_CORE - 1 else ppv
            etag = "st" if it == PER_CORE - 1 else "pv"

            def qpiece(q):
                tp = epool.tile([128, 128], MM_DT, name=f"tp{it}_{q}", tag=etag)
                nc.tensor.transpose(tp, onat[it][:, q, :], id_sb)
                if act_cp:
                    nc.scalar.activation(
                        ot[:, q * 128 : (q + 1) * 128], tp, AF.Copy
                    )
                else:
                    nc.vector.tensor_copy(ot[:, q * 128 : (q + 1) * 128], tp)
                op = epool.tile([128, D], F32, name=f"op{it}_{q}", tag=etag)
                nc.tensor.matmul(
                    op, ot[:, q * 128 : (q + 1) * 128], wo_sb, start=True, stop=True
                )
                nc.vector.tensor_add(y[:, q, :], op, xpb_sb[:, q, :])
                st6 = work.tile([128, 6], F32, name=f"st6{it}_{q}", tag="st6", bufs=8)
                bn.bn_stats(st6, y[:, q, :])
                bn.bn_aggr(mv[:, q, :], st6)

            def rstd(lo, nq):
                # rstd = 1/sqrt(var+eps) via quake + 2 Newton steps (DVE only)
                hs = slice(lo, lo + nq)
                ve = work.tile([128, nq], F32, name=f"ve{it}_{lo}", tag="ve", bufs=4)
                nc.vector.tensor_scalar_add(ve, mv[:, hs, 1], EPS)
                vh = work.tile([128, nq], F32, name=f"vh{it}_{lo}", tag="vh", bufs=4)
                nc.vector.tensor_scalar_mul(vh, ve, half_sb[:, 0:1])
                yqi = yq.bitcast(I32)[:, hs]
                nc.vector.tensor_scalar(
                    yqi, ve.bitcast(I32), 1, None, ALU.logical_shift_right
                )
                nc.vector.tensor_tensor(
                    yqi, magic_sb[:, 0:1].to_broadcast((128, nq)).bitcast(I32),
                    yqi, ALU.subtract,
                )
                t1 = work.tile([128, nq], F32, name=f"t1{it}_{lo}", tag="t1", bufs=4)
                t2 = work.tile([128, nq], F32, name=f"t2{it}_{lo}", tag="t2", bufs=4)
                yqh = yq[:, hs]
                for _ in range(2):
                    nc.vector.tensor_tensor(t1, yqh, yqh, ALU.mult)
                    nc.vector.tensor_tensor(t2, vh, t1, ALU.mult)
                    nc.vector.tensor_tensor(
                        t1, c32_sb[:, 0:1].to_broadcast((128, nq)), t2, ALU.subtract
                    )
                    nc.vector.tensor_tensor(yqh, yqh, t1, ALU.mult)

            def lnpiece(q):
                if ln_trivial:
                    lne.tensor_scalar(
                        oall[:, q, :], y[:, q, :], mv[:, q, 0:1], yq[:, q : q + 1],
                        op0=ALU.subtract, op1=ALU.mult,
                    )
                else:
                    z = work.tile([128, D], F32, name=f"z{it}_{q}", tag="z", bufs=4)
                    lne.tensor_scalar(
                        z, y[:, q, :], mv[:, q, 0:1], yq[:, q : q + 1],
                        op0=ALU.subtract, op1=ALU.mult,
                    )
                    nc.gpsimd.tensor_mul(z, z, lng_sb)
                    nc.gpsimd.tensor_add(oall[:, q, :], z, lnb_sb)
                out_dst = out_d[it].rearrange("(q p) d -> p q d", p=128)
                if it == PER_CORE - 1:
                    if q == NTILE - 1 or q == NTILE // 2 - 1:
                        nc.sync.dma_start(
                            out_dst[:, q - 3 : q + 1], oall[:, q - 3 : q + 1]
                        )
                elif q == NTILE // 2 - 1:
                    nc.sync.dma_start(out_dst[:, 0 : NTILE // 2], oall[:, 0 : NTILE // 2])
                elif q == NTILE - 1:
                    nc.sync.dma_start(
                        out_dst[:, NTILE // 2 :], oall[:, NTILE // 2 :]
                    )

            halfn = NTILE // 2
            return (
                [(lambda q=q: qpiece(q)) for q in range(halfn)]
                + [lambda: rstd(0, halfn)]
                + [(lambda q=q: lnpiece(q)) for q in range(halfn)]
                + [(lambda q=q: qpiece(q)) for q in range(halfn, NTILE)]
                + [lambda: rstd(halfn, NTILE - halfn)]
                + [(lambda q=q: lnpiece(q)) for q in range(halfn, NTILE)]
            )

        def emit_epilogue(it):
            for p in epi_pieces(it):
                p()

        # ---- software-pipelined emission over 12 (pair, head) slots ----
        # boundary work (epilogue, next-pair qk/v) is split into small
        # pieces pumped into the m-loop ahead of the (potentially blocking)
        # score-reader ops so the in-order engine queues never starve
        SLOTS = [(it, h) for it in range(PER_CORE) for h in range(H)]
        LASTS = len(SLOTS) - 1
        pending = []

        def pump(k):
            for _ in range(min(k, len(pending))):
                pending.pop(0)()

        emit_qk(0)
        pending.extend(v_pieces(0))
        for s, (it, h) in enumerate(SLOTS):
            prev = SLOTS[s - 1] if s > 0 else None
            pat = _slot_pattern(s)
            if h == 1 and it + 1 < PER_CORE:
                x_sb[it + 1] = load_x(it + 1)
            if h == 1 and it > 0:
                pending.extend(epi_pieces(it - 1))
            if h == 2 and it + 1 < PER_CORE:
                pending[0:0] = qk_pieces(it + 1) + v_pieces(it + 1)
            es_all[(it, h)] = es = []
            for m in range(NTILE):
                es.append(emit_score_m(it, h, m, pat[m]))
                pump(2 if h >= 2 else 1)
                if prev is not None and m >= 1:
                    emit_av_chunk(*prev, q=m - 1)
            if prev is not None:
                emit_av_chunk(*prev, q=NTILE - 1)
                emit_norm(*prev)
                es_all.pop(prev)
            if s == LASTS:
                # drain in m-major order so AV matmuls start as each es lands
                av_all[(it, h)] = pav.tile(
                    [128, NTILE, DH + 1], F32, name=f"av{it}_{h}", tag="av"
                )
                av = av_all[(it, h)]
                va = vaugs[it]
                for m in range(NTILE):
                    for q in range(NTILE):
                        nc.tensor.matmul(
                            av[:, q, :],
                            es[m][:, q * 128 : (q + 1) * 128],
                            va[m][:, h, :],
                            start=(m == 0),
                            stop=(m == NTILE - 1),
                        )
        pump(len(pending))
        last = SLOTS[-1]
        emit_norm(*last)
        emit_epilogue(PER_CORE - 1)

    nc.compile()
    return nc


_nc_cache = {}


def _get_nc(ln_trivial=True):
    key = (ln_trivial, PAT1, PAT2, NP1, PATL, BN_POOL, LN_POOL)
    if key not in _nc_cache:
        _nc_cache[key] = _build_nc(ln_trivial)
    return _nc_cache[key]


def make_inputs(
    node_features, adj_mx, node_type_embed, Wq, bq, Wk, bk, Wv, bv,
    edge_bias, Wo, bo, ln_g, ln_b,
):
    """Host-side prep: returns (in_maps, ln_trivial)."""
    nf = np.asarray(node_features, np.float32)
    adj = np.asarray(adj_mx)
    nte = np.asarray(node_type_embed, np.float32)
    Wq = np.asarray(Wq, np.float32)
    Wk = np.asarray(Wk, np.float32)
    Wv = np.asarray(Wv, np.float32)
    Wo = np.asarray(Wo, np.float32)
    bq = np.asarray(bq, np.float32)
    bk = np.asarray(bk, np.float32)
    bv = np.asarray(bv, np.float32)
    bo = np.asarray(bo, np.float32)
    edge_bias = np.asarray(edge_bias, np.float32)
    ln_g = np.asarray(ln_g, np.float32)
    ln_b = np.asarray(ln_b, np.float32)

    scale = 1.0 / np.sqrt(DH)
    types = 1 - (np.arange(N) % 2)
    enh = nf + nte[types][None, :, None, :]  # (B,N,T,D)
    keep = np.maximum(adj.astype(np.float32), np.eye(N, dtype=np.float32))
    ebT = edge_bias.T  # (m, q)
    keepT = keep.T
    corr_a = C1 * np.log(CORR)
    maskc2 = np.where(keepT > 0, C2_KEEP + C1 * ebT, C2_MASK)
    maska = np.where(keepT > 0, C1 * ebT + corr_a, MASKA_MASK)
    maskm = np.exp(ebT) * keepT * CORR

    ln_trivial = bool(np.all(ln_g == 1.0) and np.all(ln_b == 0.0))
    bias_trivial = bool(np.all(bq == 0.0))

    shared = {
        "wq": np.ascontiguousarray(Wq.T * (scale * C1)).astype(MM_NP),
        "wk": np.ascontiguousarray(Wk.T).astype(MM_NP),
        "wv": np.ascontiguousarray(Wv.T).astype(MM_NP),
        "wo": np.ascontiguousarray(Wo.T).astype(MM_NP),
        "bqk": np.ascontiguousarray(
            np.stack([bq * (scale * C1), bk], axis=1)
        ).astype(np.float32),
        "bvb": np.ascontiguousarray(np.broadcast_to(bv, (128, D))).astype(np.float32),
        "lng": np.ascontiguousarray(np.broadcast_to(ln_g, (128, D))),
        "lnb": np.ascontiguousarray(np.broadcast_to(ln_b, (128, D))),
        "ident": np.eye(128, dtype=MM_NP),
        "maskc2": np.ascontiguousarray(maskc2).astype(MM_NP),
        "maska": np.ascontiguousarray(maska).astype(MM_NP),
        "maskm": np.ascontiguousarray(maskm).astype(MM_NP),
    }
    in_maps = []
    for c in range(NCORES):
        pairs = PAIRS[c * PER_CORE : (c + 1) * PER_CORE]
        enhT = np.stack(
            [np.ascontiguousarray(enh[b, :, t, :].T).astype(MM_NP) for (b, t) in pairs]
        )
        xpb = np.stack([nf[b, :, t, :] + bo for (b, t) in pairs])
        in_maps.append({**shared, "enh": enhT, "xpb": xpb})
    return in_maps, ln_trivial, bias_trivial


def kernel(
    node_features, adj_mx, node_type_embed, Wq, bq, Wk, bk, Wv, bv,
    edge_bias, Wo, bo, ln_g, ln_b,
):
    global LAST_RESULTS
    in_maps, ln_trivial, bias_trivial = make_inputs(
        node_features, adj_mx, node_type_embed, Wq, bq, Wk, bk, Wv, bv,
        edge_bias, Wo, bo, ln_g, ln_b,
    )
    nc = _get_nc(ln_trivial, bias_trivial)
    res = run_bass_kernel_spmd(
        nc,
        in_maps,
        core_ids=list(range(NCORES)),
        trace=bool(int(os.environ.get("BASSK_TRACE", "0"))),
    )
    LAST_RESULTS = res

    out = np.empty((B, N, T, D), np.float32)
    for c in range(NCORES):
        pairs = PAIRS[c * PER_CORE : (c + 1) * PER_CORE]
        for i, (b, t) in enumerate(pairs):
            out[b, :, t, :] = res.results[c]["out"][i]
    return out
